# revision 4
# baseline (speedup 1.0000x reference)
"""MeshPool segment-mean kernel for Trainium2 (8 NeuronCores, SPMD).

Problem: fe [B=32, C=512, E=18000] f32, groups [B, E] int32 in [0, T=9000).
Output: [B, C, T] f32 where out[b, :, t] = mean of fe[b, :, e] over edges e
with groups[b, e] == t (empty groups -> 0).

Strategy (count-stratified gather-reduce, batch-sharded over 8 cores,
4 meshes per core):
  Host (index bookkeeping only): per mesh, bucket groups by their edge
  count c. For each count-class, build gather index tables so that the c
  member edge rows of a group land in the same SBUF partition at c
  adjacent slots.
  Device: dma_gather edge rows (2048 B each, edge-major) -> strided DVE
  adds reduce the c members -> scale by 1/c -> dma_scatter_add result
  rows to out[t] (indices unique per call, so the HBM read-modify-write
  accumulation is race-free). Groups with count > CMAX_EXACT go through
  one overflow class padded to the global max count with a per-group
  reciprocal. Empty groups are never touched (output buffer starts
  zeroed).

The Bass program is identical on all 8 cores (one NEFF, SPMD); per-core
index tables are padded to the max class size over all 32 meshes with
dummy groups (gather the zero row, scatter into a scrap row at T).
"""

import os
import numpy as np

B, C, E, T = 32, 512, 18000, 9000
NCORES = 8
MPC = B // NCORES          # meshes per core
CMAX_EXACT = 6             # exact classes 1..CMAX_EXACT, overflow above
TOK_CHUNK = 2048           # max gathered tokens per chunk (SBUF tile sizing)

# set by kernel() after a traced run (test harness support)
LAST_MODELED_NS = None


def _set_dims(b, c, e, t, ncores):
    """Debug hook: override problem dimensions (test scaffolding only)."""
    global B, C, E, T, NCORES, MPC
    B, C, E, T, NCORES = b, c, e, t, ncores
    MPC = B // NCORES


def _pad128(n):
    return max(128, ((n + 127) // 128) * 128)


def _build_mesh_tables(g_b, kov):
    """Per-mesh class tables.

    Returns {c: (members [G_c, c] int32, gids [G_c] int32)} for exact
    classes plus ("ov", members [G_ov, kov], gids, recip [G_ov] f32).
    Member value E means "dummy edge" (points at the zero row).
    """
    cnt = np.bincount(g_b, minlength=T)
    order = np.argsort(g_b, kind="stable").astype(np.int64)
    start = np.zeros(T, np.int64)
    np.cumsum(cnt[:-1], out=start[1:])

    out = {}
    for c in range(1, CMAX_EXACT + 1):
        sel = np.where(cnt == c)[0]
        if sel.size:
            m = order[start[sel][:, None] + np.arange(c)[None, :]]
        else:
            m = np.zeros((0, c), np.int64)
        out[c] = (m.astype(np.int32), sel.astype(np.int32))

    sel = np.where(cnt > CMAX_EXACT)[0]
    if sel.size:
        pos = start[sel][:, None] + np.arange(kov)[None, :]
        valid = np.arange(kov)[None, :] < cnt[sel][:, None]
        m = np.where(valid, order[np.minimum(pos, E - 1)], E)
        recip = (1.0 / cnt[sel]).astype(np.float32)
    else:
        m = np.zeros((0, kov), np.int64)
        recip = np.zeros((0,), np.float32)
    out["ov"] = (m.astype(np.int32), sel.astype(np.int32), recip)
    return out


def _class_plan(groups):
    """Global (SPMD-identical) class sizes + per-mesh tables."""
    kov = max(2, int(max(np.bincount(groups[b], minlength=T).max()
                         for b in range(B))))
    tables = [_build_mesh_tables(groups[b], kov) for b in range(B)]

    sizes = {}
    for c in range(1, CMAX_EXACT + 1):
        mx = max(t[c][0].shape[0] for t in tables)
        if mx:
            sizes[c] = _pad128(mx)
    mx = max(t["ov"][0].shape[0] for t in tables)
    if mx:
        sizes["ov"] = _pad128(mx)
    return kov, sizes, tables


def _tokens_for_class(members_padded, c):
    """members [Gpad, c] -> gather token stream: token (p*c+k)*128+r holds
    member k of group p*128+r."""
    gpad = members_padded.shape[0]
    return (
        members_padded.reshape(gpad // 128, 128, c)
        .transpose(0, 2, 1)
        .reshape(-1)
    )


def _wrap16(a):
    """Token stream -> [128, n/16] int16 (wrapped in 16 partitions,
    replicated for all 8 Q7 core groups)."""
    w = a.reshape(-1, 16).T.astype(np.int16)
    return np.tile(w, (8, 1)).copy()


def _pack_core_tables(tables_core, kov, sizes):
    """Pack per-core gather/scatter/recip arrays (fixed layout across cores).

    Returns gidx [128, GT/16] i16, sidx [128, ST/16] i16,
    recip [128, RT] f32, and the per-(mesh, class) token/group offsets.
    """
    class_list = [c for c in range(1, CMAX_EXACT + 1) if c in sizes]
    if "ov" in sizes:
        class_list.append("ov")

    gtok, stok, rcols = [], [], []
    layout = []  # (mesh, cls, tok_off, grp_off, rec_off)
    tok_off = grp_off = rec_off = 0
    for m, tab in enumerate(tables_core):
        for cls in class_list:
            gpad = sizes[cls]
            width = kov if cls == "ov" else cls
            if cls == "ov":
                mem, gid, rec = tab["ov"]
            else:
                mem, gid = tab[cls]
                rec = None
            mem_p = np.full((gpad, width), E, np.int32)
            mem_p[: mem.shape[0]] = mem
            gid_p = np.full((gpad,), T, np.int32)
            gid_p[: gid.shape[0]] = gid
            gtok.append(_tokens_for_class(mem_p, width))
            stok.append(gid_p)
            layout.append((m, cls, tok_off, grp_off, rec_off))
            tok_off += gpad * width
            grp_off += gpad
            if cls == "ov":
                rec_p = np.zeros((gpad,), np.float32)
                rec_p[: rec.shape[0]] = rec
                # group g -> partition g%128, slot g//128
                rcols.append(rec_p.reshape(gpad // 128, 128).T)
                rec_off += gpad // 128
    gidx = _wrap16(np.concatenate(gtok))
    sidx = _wrap16(np.concatenate(stok))
    recip = (
        np.concatenate(rcols, axis=1)
        if rcols
        else np.zeros((128, 1), np.float32)
    )
    return gidx, sidx, np.ascontiguousarray(recip), layout


def _build_program(kov, sizes, gidx_cols, sidx_cols, recip_cols):
    import concourse.bacc as bacc
    import concourse.mybir as mybir
    from concourse import tile

    class_list = [c for c in range(1, CMAX_EXACT + 1) if c in sizes]
    if "ov" in sizes:
        class_list.append("ov")

    nc = bacc.Bacc("TRN2", target_bir_lowering=False, debug=False,
                   num_devices=NCORES)
    fe_t = [
        nc.dram_tensor(f"fe{m}", [E + 2, C], mybir.dt.float32,
                       kind="ExternalInput")
        for m in range(MPC)
    ]
    out_t = [
        nc.dram_tensor(f"out{m}", [T + 2, C], mybir.dt.float32,
                       kind="ExternalOutput")
        for m in range(MPC)
    ]
    gidx_t = nc.dram_tensor("gidx", [128, gidx_cols], mybir.dt.int16,
                            kind="ExternalInput")
    sidx_t = nc.dram_tensor("sidx", [128, sidx_cols], mybir.dt.int16,
                            kind="ExternalInput")
    recip_t = nc.dram_tensor("recip", [128, recip_cols], mybir.dt.float32,
                             kind="ExternalInput")

    with tile.TileContext(nc) as tc:
        with (
            tc.tile_pool(name="idx", bufs=1) as idx_pool,
            tc.tile_pool(name="g", bufs=3) as g_pool,
            tc.tile_pool(name="r", bufs=3) as r_pool,
        ):
            gidx_sb = idx_pool.tile([128, gidx_cols], mybir.dt.int16)
            sidx_sb = idx_pool.tile([128, sidx_cols], mybir.dt.int16)
            recip_sb = idx_pool.tile([128, recip_cols], mybir.dt.float32)
            nc.sync.dma_start(gidx_sb[:, :], gidx_t.ap())
            nc.sync.dma_start(sidx_sb[:, :], sidx_t.ap())
            nc.sync.dma_start(recip_sb[:, :], recip_t.ap())

            limit = int(os.environ.get("MESHPOOL_LIMIT_CALLS", "0"))
            emitted = 0
            tok_off = grp_off = rec_off = 0
            for m in range(MPC):
                for cls in class_list:
                    gpad = sizes[cls]
                    width = kov if cls == "ov" else cls
                    panels_per_chunk = max(1, TOK_CHUNK // (128 * width))
                    panels = gpad // 128
                    for p0 in range(0, panels, panels_per_chunk):
                        if limit and emitted >= limit:
                            continue
                        emitted += 1
                        pn = min(panels_per_chunk, panels - p0)
                        ntok = pn * 128 * width
                        ngrp = pn * 128
                        ct = tok_off + p0 * 128 * width
                        cg = grp_off + p0 * 128

                        g_tile = g_pool.tile(
                            [128, pn * width, C], mybir.dt.float32, tag="g")
                        nc.gpsimd.dma_gather(
                            g_tile[:, :, :],
                            fe_t[m].ap(),
                            gidx_sb[:, ct // 16: (ct + ntok) // 16],
                            ntok,
                            ntok,
                            C,
                            single_packet=False,
                        )
                        if width == 1:
                            res = g_tile
                        else:
                            res = r_pool.tile(
                                [128, pn, C], mybir.dt.float32, tag="r")
                            nc.vector.tensor_add(
                                res[:, :, :],
                                g_tile[:, 0::width, :],
                                g_tile[:, 1::width, :],
                            )
                            for k in range(2, width):
                                nc.vector.tensor_add(
                                    res[:, :, :],
                                    res[:, :, :],
                                    g_tile[:, k::width, :],
                                )
                            if cls == "ov":
                                rb = recip_sb[
                                    :, rec_off + p0: rec_off + p0 + pn, None
                                ].broadcast_to([128, pn, C])
                                nc.vector.tensor_mul(
                                    res[:, :, :], res[:, :, :], rb)
                            else:
                                nc.scalar.mul(
                                    res[:, :, :], res[:, :, :], 1.0 / width)
                        nc.gpsimd.dma_scatter_add(
                            out_t[m].ap(),
                            res[:, :, :],
                            sidx_sb[:, cg // 16: (cg + ngrp) // 16],
                            ngrp,
                            ngrp,
                            C,
                            single_packet=False,
                        )
                    tok_off += gpad * width
                    grp_off += gpad
                    if cls == "ov":
                        rec_off += gpad // 128
    nc.compile()
    return nc


def kernel(fe, groups):
    global LAST_MODELED_NS
    from concourse import bass_utils

    fe = np.asarray(fe, np.float32)
    groups = np.asarray(groups)

    kov, sizes, tables = _class_plan(groups)

    # per-core packed index tables (same layout everywhere)
    packed = [
        _pack_core_tables(tables[j * MPC: (j + 1) * MPC], kov, sizes)
        for j in range(NCORES)
    ]
    gidx0, sidx0, recip0, _ = packed[0]

    nc = _build_program(kov, sizes, gidx0.shape[1], sidx0.shape[1],
                        recip0.shape[1])

    in_maps = []
    for j in range(NCORES):
        gidx, sidx, recip, _ = packed[j]
        m = {"gidx": gidx, "sidx": sidx, "recip": recip}
        for i in range(MPC):
            b = j * MPC + i
            arr = np.empty((E + 2, C), np.float32)
            arr[:E] = fe[b].T
            arr[E:] = 0.0
            m[f"fe{i}"] = arr
        in_maps.append(m)

    if os.environ.get("MESHPOOL_MODEL_TIME") == "1":
        from concourse.timeline_sim import TimelineSim

        LAST_MODELED_NS = TimelineSim(nc, no_exec=True).simulate()

    res = bass_utils.run_bass_kernel_spmd(
        nc, in_maps, core_ids=list(range(NCORES)), trace=False
    )

    out = np.empty((B, T, C), np.float32)
    for j in range(NCORES):
        for i in range(MPC):
            out[j * MPC + i] = res.results[j][f"out{i}"][:T]
    return out.transpose(0, 2, 1)
